# revision 57
# baseline (speedup 1.0000x reference)
"""DetectiveNN Trainium2 kernel: two 2-layer bidirectional LSTM stacks.

V3 layout: 8 NeuronCores, each runs ONE direction for 48 streams:
  16 streams of the `rnn` stack + 32 streams of the `rnnp` stack (the two
  speaker parties share rnnp weights, so their 128 compacted streams merge).
Core pairs (2k, 2k+1) = (fwd, bwd) over the same 48 streams; bwd cores get
time-reversed inputs so every core runs the same forward-scan program (SPMD).

Per layer each core computes its own input projection xg = Wih@x + b (bf16
GEMM) and the recurrent scan.  The IP is CHUNK-INTERLEAVED into the scan
steps: the scan's serial act/vector chain leaves the PE idle, so IP matmuls
fill those gaps, which also keeps the tensor engine in its fast p-state.
The recurrent Whh matmuls use fp8e4 + DoubleRow (two 128-K-chunks per
instruction at 0.5 cyc/row); Wih/Whh/bias are pre-scaled by 32 so fp8 hits a
good range, and the activations descale via their scale operand (1/32).
Gate order is repacked [g, i, f, o] with two PSUM stop-groups per unit so
tanh(g)/sigmoid(i) fire at 50% of the step's matmuls, sigmoid(f,o) at 100%.

L0->L1 handoff: pairwise AllGather of per-step h (bf16) in two t-halves; the
half needed first by the reversed reader ships first.  Partner h is consumed
through a reversed-t access pattern directly - no re-materialized copies.

Speaker compaction and scatter-back are host-side numpy (pure indexing).
"""

import dataclasses
import os

import ml_dtypes
import numpy as np

T, B, D, H, P = 256, 64, 1024, 512, 2
S1, S2, S = 16, 32, 48  # rnn streams, rnnp streams, total per core
DK = 8                  # contraction chunks of 128 (D=1024 and 2H=1024)
HK = 4                  # H chunks of 128
MC = 16                 # gate chunks of 128 (4H=2048)
NCORE = 8
GT = 8                  # t-steps per IP group (= steps per For_i body)
NG = T // GT            # groups per layer
GLEAD = 4               # IP groups computed ahead of the scan
LEAD_T = GLEAD * GT
TH = T // 2             # t-half for the chunked AllGather
WS = 32.0               # fp8 weight prescale
XG_PREF = 3             # xg load lookahead (steps)
BF16 = ml_dtypes.bfloat16
FP8 = ml_dtypes.float8_e4m3fn

_CACHE = {}


def _safe_tc(tile_mod, bass_rust):
    """TileContext whose tail drain splits sem waits one per instruction
    (this walrus build rejects any Drain carrying >1 sync wait)."""
    from concourse.vector_clock import ScopedClock

    class SafeTC(tile_mod.TileContext):
        def _drain_and_barrier(self, tick_clock, wait_clock):
            drain_inst = self.nc.sync.drain()
            wait_clock.add_sem_waits(
                drain_inst.ins, ScopedClock({None: tick_clock.global_clock})
            )
            di = drain_inst.ins
            if di.sync_info is None:
                self.nc.all_engine_barrier()
                popped = self.nc._tile_sem_poison_stack.pop()
                assert popped is self._sem_poison
                self.nc.clear_and_free_semaphores(
                    list(self.sems.allocated().values())
                )
                self.nc.all_engine_barrier()
                return
            waits = list(di.sync_info.on_wait)
            ups = list(di.sync_info.on_update)
            if len(waits) > 1:
                di.sync_info = bass_rust.SyncInfo(on_wait=[waits[0]], on_update=ups)
                for w in waits[1:]:
                    d2 = self.nc.sync.drain()
                    d2.ins.sync_info = bass_rust.SyncInfo(on_wait=[w], on_update=[])
            self.nc.all_engine_barrier()
            popped = self.nc._tile_sem_poison_stack.pop()
            assert popped is self._sem_poison
            self.nc.clear_and_free_semaphores(list(self.sems.allocated().values()))
            self.nc.all_engine_barrier()

    return SafeTC


def _rev_t(ap, t_dim_in_ap):
    """Reverse the t dimension of an AP in place: read last element first."""
    aps = [list(p) for p in ap.ap]
    stride, cnt = aps[t_dim_in_ap]
    aps[t_dim_in_ap][0] = -stride
    return dataclasses.replace(ap, offset=ap.offset + (cnt - 1) * stride, ap=aps)


def _split_waits(nc, mybir, limit=1):
    """This walrus build rejects instructions carrying more than one sync
    wait.  Spill excess waits onto no-op absorber instructions inserted just
    before the offender (same engine, same basic block -> same semantics)."""
    for f in nc.m.functions:
        for bb in f.blocks:
            il = bb.instructions
            out = []
            changed = False
            for inst in il:
                si = inst.sync_info
                if si is not None and len(si.on_wait) > limit:
                    waits = list(si.on_wait)
                    for w in waits[:-limit] if limit else waits:
                        out.append(mybir.InstNoOp(
                            name=nc.get_next_instruction_name(),
                            engine=inst.engine,
                            sync_info=mybir.SyncInfo(on_wait=[w], on_update=[]),
                            bass_nofuse=True,
                        ))
                    inst.sync_info = mybir.SyncInfo(
                        on_wait=waits[-limit:] if limit else [],
                        on_update=list(si.on_update),
                    )
                    changed = True
                out.append(inst)
            if changed:
                bb.instructions = out


def _loop(tc, lo, hi, step, unroll):
    """Either a hardware For_i loop or a Python unrolled loop (sim timing)."""
    from contextlib import contextmanager

    if unroll:
        @contextmanager
        def _it(i):
            yield i
        return [_it(i) for i in range(lo, hi, step)]
    return [tc.For_i(lo, hi, step, staggered_reset=False)]


class _Ctx:
    """Bundle of build-time handles shared by the emit helpers."""


def _emit_rhs_load(C, rhs, tok_base, w1, wS1, wS2):
    """Load rhs [128, DK, GT, S] for the IP group at token tok_base + w.
    For L1 (C.l1_src set): kc 0:4 <- own h (normal time), kc 4:8 <- partner
    h (already reversed by the producer); the L1 Wih K-halves are pre-swapped
    host-side for bwd cores so this layout is direction-independent."""
    nc, bass = C.nc, C.bass
    if C.l1_src is None:
        nc.sync.dma_start(
            out=rhs[:, :, :, :],
            in_=C.xT.rearrange("k p t j -> p k t j")
            [:, :, tok_base:, :][:, :, bass.ds(w1, GT), :],
        )
    else:
        # own half only; the partner half is consumed straight from the
        # SBUF granule tiles (see C.gran)
        norm_view = C.l1_src
        nc.sync.dma_start(
            out=rhs[:, 0:HK, :, :],
            in_=norm_view[:, :, bass.ds(w1, GT), :],
        )


def _emit_granule_load(C, g):
    """Load the 64-token partner-h granule g (tokens [64g, 64g+64)) from the
    gathered reversed buffer into SBUF - the only slot-dynamic reads."""
    nc = C.nc
    dt = C.mybir.dt
    tile = C.prt_pool.tile([128, HK, 64, S], dt.bfloat16)
    nc.scalar.dma_start(out=tile[:, :, :, :], in_=C.gR_v[g][:, :, :, :])
    C.gran[g] = tile


def _emit_ip_subchunk(C, j, tok_base, w1, wS1, wS2):
    """IP sub-chunk j (of GT=8) for the group at token tok_base + w:
    mc chunks (2j, 2j+1) for both units.  j==0 allocates + loads this
    group's rhs tile (pool bufs pipeline the load across groups)."""
    nc, bass, mybir = C.nc, C.bass, C.mybir
    dt = mybir.dt

    tau = tok_base + w1  # absolute first token of this group (int)
    if j == 0:
        C.ip_rhs = C.rhs_pool.tile([128, DK, GT, S], dt.bfloat16)
        _emit_rhs_load(C, C.ip_rhs, tok_base, w1, wS1, wS2)
        if C.l1_src is not None and (tau + 32) % 64 == 0 and tau + 32 < T:
            _emit_granule_load(C, (tau + 32) // 64)
    rhs = C.ip_rhs
    mco = 2 * j

    for u in (1, 0):
        if u == 1:
            ssl, su, wih_sb = slice(S1, S), S2, C.wih2_sb
        else:
            ssl, su, wih_sb = slice(0, S1), S1, C.wih1_sb
        nfree = GT * su

        for m2 in range(2):
            mc = mco + m2
            ps = C.ip_ps_pool.tile([128, 512], dt.float32, space="PSUM")
            for kc in range(DK):
                if C.l1_src is not None and kc >= HK:
                    # partner half from the 64-token SBUF granule
                    gran = C.gran[tau // 64]
                    off = tau % 64
                    moving = gran[:, kc - HK, off:off + GT, ssl]
                else:
                    moving = rhs[:, kc, :, ssl]
                nc.tensor.matmul(
                    ps[:, 0:nfree],
                    wih_sb[:, kc, mc, :],
                    moving,
                    start=(kc == 0),
                    stop=(kc == DK - 1),
                )
            C.pending_st.append((u, mc, ps, tok_base, w1))


def _emit_ip_sts(C):
    """Drain pending IP psum->xg stages (bias add, bf16, slab store).
    Emitted after the scan-step chain so these never delay it in the
    ACT/DVE queues."""
    nc, bass, mybir = C.nc, C.bass, C.mybir
    dt = mybir.dt
    for (u, mc, ps, tok_base, w1) in C.pending_st:
        if u == 1:
            su, bias_sb = S2, C.bias2_sb
        else:
            su, bias_sb = S1, C.bias1_sb
        nfree = GT * su
        if mc % 8 == 0:
            if u == 1:
                C.stg2 = C.stg2_pool.tile([128, GT, 8, S2], dt.bfloat16)
            else:
                C.stg1 = C.stg1_pool.tile([128, GT, 8, S1], dt.bfloat16)
        stg = C.stg2 if u == 1 else C.stg1
        nc.vector.tensor_scalar(
            stg[:, :, mc % 8, :],
            ps[:, 0:nfree].rearrange("p (t j) -> p t j", t=GT),
            bias_sb[:, mc:mc + 1],
            None,
            mybir.AluOpType.add,
        )
        if mc % 8 == 7:  # slab of 8 mc chunks complete -> store
            slab = mc - 7
            xg_u = C.xg2 if u == 1 else C.xg1
            nc.sync.dma_start(
                out=xg_u[:, tok_base:, slab:slab + 8, :][:, bass.ds(w1, GT), :, :],
                in_=stg[:, :, :, :],
            )
    C.pending_st = []


def _emit_scan_pre(C, j, tok_base, w1):
    """xg loads + PSUM-claiming ident injections for step tok_base + w + j.
    These have no dependence on h, so they run while the previous step's
    activation chain completes."""
    nc, bass, mybir = C.nc, C.bass, C.mybir
    dt = mybir.dt

    if j % 4 == 0:
        # quad xg load: 4 steps per DMA
        C.xgt1 = C.xgt1_pool.tile([128, 4, MC, S1], dt.bfloat16)
        C.xgt2 = C.xgt2_pool.tile([128, 4, MC, S2], dt.bfloat16)
        nc.sync.dma_start(
            out=C.xgt2[:, :, :, :],
            in_=C.xg2[:, tok_base + j:, :, :][:, bass.ds(w1, 4), :, :],
        )
        nc.sync.dma_start(
            out=C.xgt1[:, :, :, :],
            in_=C.xg1[:, tok_base + j:, :, :][:, bass.ds(w1, 4), :, :],
        )
    for u in (1, 0):
        if u == 1:
            gA, gB, xgt, su = C.g2A, C.g2B, C.xgt2, S2
        else:
            gA, gB, xgt, su = C.g1A, C.g1B, C.xgt1, S1
        for grp, gps in ((0, gA), (1, gB)):
            for m8 in range(8):
                nc.tensor.matmul(
                    gps[:, m8, 0:su],
                    C.ident_sb[:, :],
                    xgt[:, j % 4, grp * 8 + m8, :],
                    start=(m8 == 0),
                    stop=False,
                )


def _emit_scan_main(C, j, tok_base, w1, hdst_view):
    """Whh matmuls + cell update for both units; h store every 2nd step.
    hdst_view: dram view [128, HK, nt, S] receiving bf16 h at index w+j."""
    nc, bass, mybir = C.nc, C.bass, C.mybir
    A = mybir.ActivationFunctionType

    for u in (1, 0):  # big unit first
        if u == 1:
            ssl, su, whh = slice(S1, S), S2, C.whh2_sb
            gA, gB, act = C.g2A, C.g2B, C.act2
            tg, t1s, t2s, tcv = C.tg2, C.t12, C.t22, C.tc2
        else:
            ssl, su, whh = slice(0, S1), S1, C.whh1_sb
            gA, gB, act = C.g1A, C.g1B, C.act1
            tg, t1s, t2s, tcv = C.tg1, C.t11, C.t21, C.tc1

        # accumulate Whh@h in fp8 DoubleRow on top of the injected xg
        for grp, gps in ((0, gA), (1, gB)):
            for m8 in range(8):
                for kp in range(2):
                    nc.tensor.matmul(
                        gps[:, m8, 0:su],
                        whh[:, kp, :, grp * 8 + m8, :],
                        C.h8[:, 2 * kp:2 * kp + 2, ssl],
                        start=False,
                        stop=(m8 == 7 and kp == 1),
                        perf_mode=mybir.MatmulPerfMode.DoubleRow,
                    )
        # group A done -> tanh(g) [chunks 0:4], sigmoid(i) [4:8]
        nc.scalar.activation(tg[:, :, :], gA[:, 0:4, 0:su], A.Tanh, scale=1.0 / WS)
        nc.scalar.activation(act[:, 0:4, :], gA[:, 4:8, 0:su], A.Sigmoid, scale=1.0 / WS)
        # group B done -> sigmoid(f,o) [8:16]
        nc.scalar.activation(act[:, 4:12, :], gB[:, :, 0:su], A.Sigmoid, scale=1.0 / WS)
        # c = sig(f)*c + sig(i)*tanh(g);  h = sig(o)*tanh(c)
        nc.gpsimd.tensor_mul(t1s[:, :, :], act[:, 0:4, :], tg[:, :, :])
        nc.vector.tensor_mul(t2s[:, :, :], act[:, 4:8, :], C.c_sb[:, :, ssl])
        nc.vector.tensor_add(C.c_sb[:, :, ssl], t1s[:, :, :], t2s[:, :, :])
        nc.scalar.activation(tcv[:, :, :], C.c_sb[:, :, ssl], A.Tanh)
        nc.vector.tensor_mul(C.h8[:, :, ssl], act[:, 8:12, :], tcv[:, :, :])
        # bf16 h for the handoff / output, from the fp32 operands (NOT from
        # the fp8 state - fp8 noise here would leak into L1 and the output);
        # 4 rotating t-slots so the paired store never stalls the queues
        nc.vector.tensor_mul(C.hbf[:, :, j % 4, ssl], act[:, 8:12, :], tcv[:, :, :])

    if j % 2 == 1:  # store two steps of h per DMA
        sl = (j - 1) % 4
        nc.gpsimd.dma_start(
            out=hdst_view[:, :, j - 1:, :][:, :, bass.ds(w1, 2), :],
            in_=C.hbf[:, :, sl:sl + 2, :],
        )


def build_nc(n_cores=NCORE, unroll=False):
    import bass_rust
    import concourse.bass as bass
    import concourse.mybir as mybir
    from concourse import tile
    from contextlib import ExitStack

    dt = mybir.dt
    nc = bass.Bass("TRN2", target_bir_lowering=False, debug=False,
                   num_devices=(1 if unroll else n_cores))

    C = _Ctx()
    C.nc, C.bass, C.mybir = nc, bass, mybir

    C.xT = nc.dram_tensor("xT", [DK, 128, T, S], dt.bfloat16, kind="ExternalInput").ap()
    w_in = {}
    for nm in ("A1", "A2", "B1", "B2"):
        w_in[f"wih{nm}"] = nc.dram_tensor(f"wih{nm}", [128, DK, MC, 128], dt.bfloat16, kind="ExternalInput").ap()
        w_in[f"whh{nm}"] = nc.dram_tensor(f"whh{nm}", [128, 2, 2, MC, 128], dt.float8e4, kind="ExternalInput").ap()
        w_in[f"bias{nm}"] = nc.dram_tensor(f"bias{nm}", [128, MC], dt.float32, kind="ExternalInput").ap()
    ident = nc.dram_tensor("ident", [128, 128], dt.bfloat16, kind="ExternalInput").ap()
    flag = nc.dram_tensor("flag", [1, 4], dt.int32, kind="ExternalInput").ap()
    out = nc.dram_tensor("out", [HK, 128, T, S], dt.bfloat16, kind="ExternalOutput").ap()

    C.xg1 = nc.dram_tensor("xg1", [128, T, MC, S1], dt.bfloat16).ap()
    C.xg2 = nc.dram_tensor("xg2", [128, T, MC, S2], dt.bfloat16).ap()
    hT = nc.dram_tensor("hT", [HK, 128, T, S], dt.bfloat16).ap()
    # own h reversed in 64-token quarters: hTr[q] = reverse(hT[T-64(q+1) :
    # T-64q]) = global tokens in descending order; quarter 0 ships first
    # (the partner's L1-IP consumes it first).
    hTr = [nc.dram_tensor(f"hTr{q}", [HK, 128, 64, S], dt.bfloat16).ap()
           for q in range(4)]
    gathR = [nc.dram_tensor(f"gathR{q}", [2, HK, 128, 64, S], dt.bfloat16).ap()
             for q in range(4)]

    SafeTC = _safe_tc(tile, bass_rust)
    groups = [[2 * k, 2 * k + 1] for k in range(max(n_cores // 2, 1))]

    with SafeTC(nc) as tc, ExitStack() as ctx:
        cpool = ctx.enter_context(tc.tile_pool(name="const", bufs=1))
        C.wih1_sb = cpool.tile([128, DK, MC, 128], dt.bfloat16, name="wih1_sb")
        C.wih2_sb = cpool.tile([128, DK, MC, 128], dt.bfloat16, name="wih2_sb")
        C.whh1_sb = cpool.tile([128, 2, 2, MC, 128], dt.float8e4, name="whh1_sb")
        C.whh2_sb = cpool.tile([128, 2, 2, MC, 128], dt.float8e4, name="whh2_sb")
        C.bias1_sb = cpool.tile([128, MC], dt.float32, name="bias1_sb")
        C.bias2_sb = cpool.tile([128, MC], dt.float32, name="bias2_sb")
        C.ident_sb = cpool.tile([128, 128], dt.bfloat16, name="ident_sb")
        flag_sb = cpool.tile([1, 4], dt.int32, name="flag_sb")

        def load_layer(nm1, nm2):
            for sb, dr in [(C.wih1_sb, w_in[f"wih{nm1}"]), (C.wih2_sb, w_in[f"wih{nm2}"]),
                           (C.whh1_sb, w_in[f"whh{nm1}"]), (C.whh2_sb, w_in[f"whh{nm2}"]),
                           (C.bias1_sb, w_in[f"bias{nm1}"]), (C.bias2_sb, w_in[f"bias{nm2}"])]:
                nc.sync.dma_start(out=sb[...], in_=dr[...])

        load_layer("A1", "A2")
        nc.sync.dma_start(out=C.ident_sb[...], in_=ident[...])
        nc.sync.dma_start(out=flag_sb[...], in_=flag[...])

        if unroll:
            C.vb = 1
        else:
            tmp = nc.alloc_registers("vb_r")
            nc.regs_load(tmp, flag_sb[0:1, 1:2])
            C.vb = nc.snap(tmp, donate=True, min_val=0, max_val=1)

        C.pending_st = []
        spool = ctx.enter_context(tc.tile_pool(name="state", bufs=1))
        C.h8 = spool.tile([128, HK, S], dt.float8e4, name="h8")
        C.hbf = spool.tile([128, HK, 4, S], dt.bfloat16, name="hbf")
        C.c_sb = spool.tile([128, HK, S], dt.float32, name="c_sb")

        def gview(gh, slot):
            """[128, HK, TH, S] view of gath half gh at slot (static)."""
            return gh[slot].rearrange("k p t j -> p k t j")

        def hview(ht):
            return ht.rearrange("k p t j -> p k t j")

        for lay in range(2):
            nm1, nm2 = ("A1", "A2") if lay == 0 else ("B1", "B2")
            with ExitStack() as phase:
                C.rhs_pool = phase.enter_context(tc.tile_pool(name=f"rhs{lay}", bufs=2))
                C.ip_ps_pool = phase.enter_context(tc.tile_pool(name=f"ipps{lay}", bufs=4, space="PSUM"))
                C.stg1_pool = phase.enter_context(tc.tile_pool(name=f"s1p{lay}", bufs=2))
                C.stg2_pool = phase.enter_context(tc.tile_pool(name=f"s2p{lay}", bufs=2))
                C.xgt1_pool = phase.enter_context(tc.tile_pool(name=f"x1p{lay}", bufs=4))
                C.xgt2_pool = phase.enter_context(tc.tile_pool(name=f"x2p{lay}", bufs=4))
                C.prt_pool = phase.enter_context(tc.tile_pool(name=f"prt{lay}", bufs=2))
                gpool = phase.enter_context(tc.tile_pool(name=f"g{lay}", bufs=1, space="PSUM"))
                apool = phase.enter_context(tc.tile_pool(name=f"act{lay}", bufs=1))

                C.g1A = gpool.tile([128, 8, 64], dt.float32, name=f"g1A{lay}", space="PSUM")
                C.g1B = gpool.tile([128, 8, 64], dt.float32, name=f"g1B{lay}", space="PSUM")
                C.g2A = gpool.tile([128, 8, 64], dt.float32, name=f"g2A{lay}", space="PSUM")
                C.g2B = gpool.tile([128, 8, 64], dt.float32, name=f"g2B{lay}", space="PSUM")
                C.act1 = apool.tile([128, 12, S1], dt.float32, name=f"a1{lay}")
                C.act2 = apool.tile([128, 12, S2], dt.float32, name=f"a2{lay}")
                C.tg1 = apool.tile([128, HK, S1], dt.float32, name=f"tg1{lay}")
                C.tg2 = apool.tile([128, HK, S2], dt.float32, name=f"tg2{lay}")
                C.t11 = apool.tile([128, HK, S1], dt.float32, name=f"t11{lay}")
                C.t12 = apool.tile([128, HK, S2], dt.float32, name=f"t12{lay}")
                C.t21 = apool.tile([128, HK, S1], dt.float32, name=f"t21{lay}")
                C.t22 = apool.tile([128, HK, S2], dt.float32, name=f"t22{lay}")
                C.tc1 = apool.tile([128, HK, S1], dt.float32, name=f"tc1{lay}")
                C.tc2 = apool.tile([128, HK, S2], dt.float32, name=f"tc2{lay}")

                if lay == 1:
                    load_layer(nm1, nm2)
                nc.vector.memset(C.h8[:, :, :], 0.0)
                nc.vector.memset(C.hbf[:, :, :, :], 0.0)
                nc.vector.memset(C.c_sb[:, :, :], 0.0)

                # segments: (nsteps, tok_base, l1 normal view, l1 rev view,
                #            h-store view, do_ip)
                if lay == 0:
                    segs = [
                        (TH, 0, None, hview(hT), True),
                        (96, TH, None, hview(hT)[:, :, TH:, :], True),
                        (LEAD_T, T - LEAD_T, None,
                         hview(hT)[:, :, T - LEAD_T:, :], False),
                    ]
                else:
                    # partner h (already reversed to global 255-tau order)
                    C.gR_v = [g[bass.ds(C.vb, 1), :, :, :, :][0]
                              .rearrange("k p t j -> p k t j") for g in gathR]
                    C.gran = {}
                    segs = [
                        (96, 0, hview(hT)[:, :, LEAD_T:, :], hview(out), True),
                        (128, 96, hview(hT)[:, :, TH:, :],
                         hview(out)[:, :, 96:, :], True),
                        (LEAD_T, T - LEAD_T, None,
                         hview(out)[:, :, T - LEAD_T:, :], False),
                    ]

                # IP lead groups (static indices; unsliced views)
                if lay == 0:
                    C.l1_src = None
                else:
                    C.l1_src = hview(hT)
                    _emit_granule_load(C, 0)
                for g in range(GLEAD):
                    base = g * GT
                    for j in range(GT):
                        _emit_ip_subchunk(C, j, 0, base, base * S1, base * S2)
                        _emit_ip_sts(C)

                def emit_rev(q):
                    # hTr[q][j] = hT[T - 64q - 1 - j]
                    lo = T - 64 * (q + 1)
                    for kc in range(HK):
                        nc.sync.dma_start(
                            out=hTr[q][kc, :, :, :],
                            in_=_rev_t(hT[kc, :, lo:lo + 64, :], 1),
                        )

                for si, (nsteps, tok_base, l1s, hv, do_ip) in enumerate(segs):
                    C.l1_src = l1s
                    for i in range(0, nsteps, GT):
                        for j in range(GT):
                            _emit_scan_pre(C, j, tok_base, i)
                            if do_ip:
                                _emit_ip_subchunk(
                                    C, j, tok_base + LEAD_T, i, i * S1, i * S2)
                            _emit_scan_main(C, j, tok_base, i, hv)
                            if do_ip:
                                _emit_ip_sts(C)
                    if lay == 0 and si == 0:
                        # first t-half of hT complete: reverse quarters 3,2 now
                        emit_rev(3)
                        emit_rev(2)

                if lay == 0:
                    # quarters 0,1 (global tokens 255..128) depend on the
                    # last scan steps; 2,3 were already reversed after seg0.
                    for q in (1, 0):
                        emit_rev(q)
                    for q in range(4):
                        if unroll:
                            for v in range(2):
                                nc.sync.dma_start(
                                    out=gathR[q][v, :, :, :, :], in_=hTr[q][...])
                        else:
                            nc.gpsimd.collective_compute(
                                "AllGather", mybir.AluOpType.bypass,
                                replica_groups=groups,
                                ins=[hTr[q][...]],
                                outs=[gathR[q][0:2, :, :, :, :]],
                            )
    _split_waits(nc, mybir)
    return nc


# ---------------- host-side data prep ----------------

GATE_PERM = (2, 0, 1, 3)  # new block order [g, i, f, o] from [i, f, g, o]


def _reorder_gates(W):
    """W: (4H, ...) -> gate blocks reordered to [g, i, f, o]."""
    blocks = W.reshape(4, H, *W.shape[1:])
    return np.concatenate([blocks[p] for p in GATE_PERM], axis=0)


def _wih_tiles(W, swap_k_halves=False):
    """(2048, K) fp32 -> [128(kp), K/128, MC, 128(mp)] bf16, x32 scaled.
    swap_k_halves: put columns 512:1024 first (bwd cores' L1 weights: the
    rhs always loads own-direction h into kc 0:4)."""
    Wr = _reorder_gates(W) * WS
    if swap_k_halves:
        Wr = np.concatenate([Wr[:, H:], Wr[:, :H]], axis=1)
    M, K = Wr.shape
    t = Wr.reshape(MC, 128, K // 128, 128)       # [mc, mp, kc, kp]
    return np.ascontiguousarray(t.transpose(3, 2, 0, 1)).astype(BF16)


def _whh_dr_tiles(W):
    """(2048, 512) fp32 -> [128(kp), 2(pair), 2(j), MC, 128(mp)] fp8, x32."""
    Wr = _reorder_gates(W) * WS
    t = Wr.reshape(MC, 128, 4, 128)              # [mc, mp, kchunk, kp]
    t = t.reshape(MC, 128, 2, 2, 128)            # [mc, mp, pair, j, kp]
    return np.ascontiguousarray(t.transpose(4, 2, 3, 0, 1)).astype(FP8)


def _bias_tiles(b):
    br = _reorder_gates(b) * WS
    return np.ascontiguousarray(br.reshape(MC, 128).T.astype(np.float32))


def _core_inputs(u1, u2):
    """u1 (T,S1,D), u2 (T,S2,D) fp32 local time -> xT (DK, 128, T, S) bf16."""
    x48 = np.concatenate([u1, u2], axis=1)          # (T, S, D)
    xt = x48.transpose(2, 0, 1)                     # (D, T, S)
    return np.ascontiguousarray(xt.reshape(DK, 128, T, S)).astype(BF16)


def _prep_inputs(inputs):
    U = np.asarray(inputs["U"], np.float32)            # (T, B, D)
    qmask = np.asarray(inputs["qmask"], np.float32)    # (B, T, P)
    U_bt = U.transpose(1, 0, 2)
    mask = qmask > 0
    pos = np.cumsum(mask.astype(np.int64), axis=1) - 1

    parties = np.zeros((P, B, T, D), np.float32)
    for p in range(P):
        b_idx, t_idx = np.nonzero(mask[:, :, p])
        parties[p, b_idx, pos[b_idx, t_idx, p]] = U_bt[b_idx, t_idx]
    partiesM = parties.reshape(P * B, T, D).transpose(1, 0, 2)  # (T, 128, D)

    def wset(stack, lay, d):
        return (
            _wih_tiles(np.asarray(inputs[f"{stack}_Wih{lay}"][d], np.float32),
                       swap_k_halves=(lay == 1 and d == 1)),
            _whh_dr_tiles(np.asarray(inputs[f"{stack}_Whh{lay}"][d], np.float32)),
            _bias_tiles(np.asarray(inputs[f"{stack}_b{lay}"][d], np.float32)),
        )

    wsets = {(st, la, d): wset(st, la, d)
             for st in ("rnn", "rnnp") for la in (0, 1) for d in (0, 1)}
    ident_np = np.eye(128, dtype=BF16)

    in_maps = []
    for c in range(NCORE):
        k, d = c // 2, c % 2
        u1 = U[:, 16 * k:16 * k + 16, :]
        u2 = partiesM[:, 32 * k:32 * k + 32, :]
        if d == 1:
            u1, u2 = u1[::-1], u2[::-1]
        m = {
            "xT": _core_inputs(u1, u2),
            "ident": ident_np,
            # [unused, partner-slot, cond_slot0=(slot==0), cond_slot1]
            "flag": np.array([[0, 1 - d, d, 1 - d]], np.int32),
        }
        for la, nm in ((0, "A"), (1, "B")):
            for ui, st in ((1, "rnn"), (2, "rnnp")):
                wih, whh, bias = wsets[(st, la, d)]
                m[f"wih{nm}{ui}"] = wih
                m[f"whh{nm}{ui}"] = whh
                m[f"bias{nm}{ui}"] = bias
        in_maps.append(m)
    return in_maps, mask, pos


def _assemble(results, mask, pos):
    # per-core out: (HK, 128, T, S) bf16; feature dim on (HK,128)=512
    o = []
    for c in range(NCORE):
        oc = np.asarray(results[c]["out"]).astype(np.float32)
        oc = oc.reshape(H, T, S).transpose(1, 2, 0)    # (T, S, 512) local time
        if c % 2 == 1:
            oc = oc[::-1]
        o.append(oc)

    U_s = np.zeros((T, B, 2 * H), np.float32)
    E = np.zeros((P, B, T, 2 * H), np.float32)
    for k in range(4):
        fwd, bwd = o[2 * k], o[2 * k + 1]
        U_s[:, 16 * k:16 * k + 16, 0:H] = fwd[:, 0:S1]
        U_s[:, 16 * k:16 * k + 16, H:2 * H] = bwd[:, 0:S1]
        for i in range(S2):
            ms = 32 * k + i
            p, b = divmod(ms, B)
            E[p, b, :, 0:H] = fwd[:, S1 + i]
            E[p, b, :, H:2 * H] = bwd[:, S1 + i]

    U_p = np.zeros((B, T, 2 * H), np.float32)
    for p in range(P):
        idx = np.clip(pos[:, :, p], 0, T - 1)
        gathered = np.take_along_axis(E[p], idx[:, :, None], axis=1)
        U_p = np.where(mask[:, :, p][:, :, None], gathered, U_p)
    U_p = U_p.transpose(1, 0, 2)
    return np.concatenate([U_s, U_p], axis=-1).astype(np.float32)


def _get_compiled():
    if "nc" not in _CACHE:
        _CACHE["nc"] = build_nc()
    return _CACHE["nc"]


def kernel(**inputs):
    from concourse.bass_utils import run_bass_kernel_spmd

    nc = _get_compiled()
    in_maps, mask, pos = _prep_inputs(inputs)
    trace = bool(int(os.environ.get("KERNEL_TRACE", "0")))
    res = run_bass_kernel_spmd(nc, in_maps, list(range(NCORE)), trace=trace)
    _CACHE["last_exec_time_ns"] = res.exec_time_ns
    return _assemble(res.results, mask, pos)


# revision 65
# speedup vs baseline: 1.0219x; 1.0219x over previous
"""DetectiveNN Trainium2 kernel: two 2-layer bidirectional LSTM stacks.

V3 layout: 8 NeuronCores, each runs ONE direction for 48 streams:
  16 streams of the `rnn` stack + 32 streams of the `rnnp` stack (the two
  speaker parties share rnnp weights, so their 128 compacted streams merge).
Core pairs (2k, 2k+1) = (fwd, bwd) over the same 48 streams; bwd cores get
time-reversed inputs so every core runs the same forward-scan program (SPMD).

Per layer each core computes its own input projection xg = Wih@x + b (bf16
GEMM) and the recurrent scan.  The IP is CHUNK-INTERLEAVED into the scan
steps: the scan's serial act/vector chain leaves the PE idle, so IP matmuls
fill those gaps, which also keeps the tensor engine in its fast p-state.
The recurrent Whh matmuls use fp8e4 + DoubleRow (two 128-K-chunks per
instruction at 0.5 cyc/row); Wih/Whh/bias are pre-scaled by 32 so fp8 hits a
good range, and the activations descale via their scale operand (1/32).
Gate order is repacked [g, i, f, o] with two PSUM stop-groups per unit so
tanh(g)/sigmoid(i) fire at 50% of the step's matmuls, sigmoid(f,o) at 100%.

L0->L1 handoff: pairwise AllGather of per-step h (bf16) in two t-halves; the
half needed first by the reversed reader ships first.  Partner h is consumed
through a reversed-t access pattern directly - no re-materialized copies.

Speaker compaction and scatter-back are host-side numpy (pure indexing).
"""

import dataclasses
import os

import ml_dtypes
import numpy as np

T, B, D, H, P = 256, 64, 1024, 512, 2
S1, S2, S = 16, 32, 48  # rnn streams, rnnp streams, total per core
DK = 8                  # contraction chunks of 128 (D=1024 and 2H=1024)
HK = 4                  # H chunks of 128
MC = 16                 # gate chunks of 128 (4H=2048)
NCORE = 8
GT = 8                  # t-steps per IP group (= steps per For_i body)
NG = T // GT            # groups per layer
GLEAD = 4               # IP groups computed ahead of the scan
LEAD_T = GLEAD * GT
TH = T // 2             # t-half for the chunked AllGather
WS = 32.0               # fp8 weight prescale
T2 = 256                # u2 (rnnp) tokens with real L0-IP work; columns
                        # >= T2 are all-pad (max party length 145 + margin),
                        # their xg is exactly the (scaled) bias
XG_PREF = 3             # xg load lookahead (steps)
BF16 = ml_dtypes.bfloat16
FP8 = ml_dtypes.float8_e4m3fn

_CACHE = {}


def _safe_tc(tile_mod, bass_rust):
    """TileContext whose tail drain splits sem waits one per instruction
    (this walrus build rejects any Drain carrying >1 sync wait)."""
    from concourse.vector_clock import ScopedClock

    class SafeTC(tile_mod.TileContext):
        def _drain_and_barrier(self, tick_clock, wait_clock):
            drain_inst = self.nc.sync.drain()
            wait_clock.add_sem_waits(
                drain_inst.ins, ScopedClock({None: tick_clock.global_clock})
            )
            di = drain_inst.ins
            if di.sync_info is None:
                self.nc.all_engine_barrier()
                popped = self.nc._tile_sem_poison_stack.pop()
                assert popped is self._sem_poison
                self.nc.clear_and_free_semaphores(
                    list(self.sems.allocated().values())
                )
                self.nc.all_engine_barrier()
                return
            waits = list(di.sync_info.on_wait)
            ups = list(di.sync_info.on_update)
            if len(waits) > 1:
                di.sync_info = bass_rust.SyncInfo(on_wait=[waits[0]], on_update=ups)
                for w in waits[1:]:
                    d2 = self.nc.sync.drain()
                    d2.ins.sync_info = bass_rust.SyncInfo(on_wait=[w], on_update=[])
            self.nc.all_engine_barrier()
            popped = self.nc._tile_sem_poison_stack.pop()
            assert popped is self._sem_poison
            self.nc.clear_and_free_semaphores(list(self.sems.allocated().values()))
            self.nc.all_engine_barrier()

    return SafeTC


def _rev_t(ap, t_dim_in_ap):
    """Reverse the t dimension of an AP in place: read last element first."""
    aps = [list(p) for p in ap.ap]
    stride, cnt = aps[t_dim_in_ap]
    aps[t_dim_in_ap][0] = -stride
    return dataclasses.replace(ap, offset=ap.offset + (cnt - 1) * stride, ap=aps)


def _split_waits(nc, mybir, limit=1):
    """This walrus build rejects instructions carrying more than one sync
    wait.  Spill excess waits onto no-op absorber instructions inserted just
    before the offender (same engine, same basic block -> same semantics)."""
    for f in nc.m.functions:
        for bb in f.blocks:
            il = bb.instructions
            out = []
            changed = False
            for inst in il:
                si = inst.sync_info
                if si is not None and len(si.on_wait) > limit:
                    waits = list(si.on_wait)
                    for w in waits[:-limit] if limit else waits:
                        out.append(mybir.InstNoOp(
                            name=nc.get_next_instruction_name(),
                            engine=inst.engine,
                            sync_info=mybir.SyncInfo(on_wait=[w], on_update=[]),
                            bass_nofuse=True,
                        ))
                    inst.sync_info = mybir.SyncInfo(
                        on_wait=waits[-limit:] if limit else [],
                        on_update=list(si.on_update),
                    )
                    changed = True
                out.append(inst)
            if changed:
                bb.instructions = out


def _loop(tc, lo, hi, step, unroll):
    """Either a hardware For_i loop or a Python unrolled loop (sim timing)."""
    from contextlib import contextmanager

    if unroll:
        @contextmanager
        def _it(i):
            yield i
        return [_it(i) for i in range(lo, hi, step)]
    return [tc.For_i(lo, hi, step, staggered_reset=False)]


class _Ctx:
    """Bundle of build-time handles shared by the emit helpers."""


def _emit_rhs_load(C, rhs, tok_base, w1, wS1, wS2):
    """Load rhs [128, DK, GT, S] for the IP group at token tok_base + w.
    For L1 (C.l1_src set): kc 0:4 <- own h (normal time), kc 4:8 <- partner
    h (already reversed by the producer); the L1 Wih K-halves are pre-swapped
    host-side for bwd cores so this layout is direction-independent."""
    nc, bass = C.nc, C.bass
    if C.l1_src is None:
        if tok_base + w1 >= C.u2_pad_from:
            # all-pad for u2: only the rnn streams need input data
            nc.scalar.dma_start(
                out=rhs[:, :, :, :],
                in_=C.xT1o.rearrange("k p t j -> p k t j")
                [:, :, tok_base:, :][:, :, bass.ds(w1, GT), :],
            )
        else:
            nc.scalar.dma_start(
                out=rhs[:, :, :, :],
                in_=C.xT.rearrange("k p t j -> p k t j")
                [:, :, tok_base:, :][:, :, bass.ds(w1, GT), :],
            )
    else:
        # own half only; the partner half is consumed straight from the
        # SBUF granule tiles (see C.gran)
        norm_view = C.l1_src
        nc.scalar.dma_start(
            out=rhs[:, 0:HK, :, :],
            in_=norm_view[:, :, bass.ds(w1, GT), :],
        )


def _emit_granule_load(C, g):
    """Load the 64-token partner-h granule g (tokens [64g, 64g+64)) from the
    gathered reversed buffer into SBUF - the only slot-dynamic reads."""
    nc = C.nc
    dt = C.mybir.dt
    tile = C.prt_pool.tile([128, HK, 64, S], dt.bfloat16)
    nc.scalar.dma_start(out=tile[:, :, :, :], in_=C.gR_v[g][:, :, :, :])
    C.gran[g] = tile


def _emit_ip_subchunk(C, j, tok_base, w1, wS1, wS2):
    """IP sub-chunk j (of GT=8) for the group at token tok_base + w:
    mc chunks (2j, 2j+1) for both units.  j==0 allocates + loads this
    group's rhs tile (pool bufs pipeline the load across groups)."""
    nc, bass, mybir = C.nc, C.bass, C.mybir
    dt = mybir.dt

    tau = tok_base + w1  # absolute first token of this group (int)
    if j == 0:
        if C.l1_src is None and tau >= C.u2_pad_from:
            # u2 all-pad: narrow rhs tile, rnn streams only
            C.ip_rhs = C.rhs1_pool.tile([128, DK, GT, S1], dt.bfloat16)
        else:
            C.ip_rhs = C.rhs_pool.tile([128, DK, GT, S], dt.bfloat16)
        _emit_rhs_load(C, C.ip_rhs, tok_base, w1, wS1, wS2)
        if C.l1_src is not None and (tau + 32) % 64 == 0 and tau + 32 < T:
            _emit_granule_load(C, (tau + 32) // 64)
    rhs = C.ip_rhs
    mco = 2 * j

    for u in (1, 0):
        if u == 1:
            if tau >= C.u2_pad_from:
                continue
            ssl, su, wih_sb = slice(S1, S), S2, C.wih2_sb
        else:
            ssl, su, wih_sb = slice(0, S1), S1, C.wih1_sb
        nfree = GT * su

        for m2 in range(2):
            mc = mco + m2
            ps = C.ip_ps_pool.tile([128, 512], dt.float32, space="PSUM")
            for kc in range(DK):
                if C.l1_src is not None and kc >= HK:
                    # partner half from the 64-token SBUF granule
                    gran = C.gran[tau // 64]
                    off = tau % 64
                    moving = gran[:, kc - HK, off:off + GT, ssl]
                else:
                    moving = rhs[:, kc, :, ssl]
                nc.tensor.matmul(
                    ps[:, 0:nfree],
                    wih_sb[:, kc, mc, :],
                    moving,
                    start=(kc == 0),
                    stop=(kc == DK - 1),
                )
            C.pending_st.append((u, mc, ps, tok_base, w1))


def _emit_ip_sts(C):
    """Drain pending IP psum->xg stages (bias add, bf16, slab store).
    Emitted after the scan-step chain so these never delay it in the
    ACT/DVE queues."""
    nc, bass, mybir = C.nc, C.bass, C.mybir
    dt = mybir.dt
    for (u, mc, ps, tok_base, w1) in C.pending_st:
        if u == 1:
            su, bias_sb = S2, C.bias2_sb
        else:
            su, bias_sb = S1, C.bias1_sb
        nfree = GT * su
        if mc % 8 == 0:
            if u == 1:
                C.stg2 = C.stg2_pool.tile([128, GT, 8, S2], dt.bfloat16)
            else:
                C.stg1 = C.stg1_pool.tile([128, GT, 8, S1], dt.bfloat16)
        stg = C.stg2 if u == 1 else C.stg1
        if u == 1:
            nc.vector.tensor_scalar(
                stg[:, :, mc % 8, :],
                ps[:, 0:nfree].rearrange("p (t j) -> p t j", t=GT),
                bias_sb[:, mc:mc + 1],
                None,
                mybir.AluOpType.add,
            )
        else:
            nc.scalar.activation(
                stg[:, :, mc % 8, :],
                ps[:, 0:nfree].rearrange("p (t j) -> p t j", t=GT),
                mybir.ActivationFunctionType.Identity,
                bias=bias_sb[:, mc:mc + 1],
            )
        if mc % 8 == 7:  # slab of 8 mc chunks complete -> store
            slab = mc - 7
            xg_u = C.xg2 if u == 1 else C.xg1
            nc.sync.dma_start(
                out=xg_u[:, tok_base:, slab:slab + 8, :][:, bass.ds(w1, GT), :, :],
                in_=stg[:, :, :, :],
            )
    C.pending_st = []


def _emit_scan_pre(C, j, tok_base, w1):
    """xg loads + PSUM-claiming ident injections for step tok_base + w + j.
    These have no dependence on h, so they run while the previous step's
    activation chain completes."""
    nc, bass, mybir = C.nc, C.bass, C.mybir
    dt = mybir.dt

    pad2 = (tok_base + w1 + j) >= C.u2_pad_from
    if j % 4 == 0:
        # quad xg load: 4 steps per DMA
        C.xgt1 = C.xgt1_pool.tile([128, 4, MC, S1], dt.bfloat16)
        nc.sync.dma_start(
            out=C.xgt1[:, :, :, :],
            in_=C.xg1[:, tok_base + j:, :, :][:, bass.ds(w1, 4), :, :],
        )
        if not pad2:
            C.xgt2 = C.xgt2_pool.tile([128, 4, MC, S2], dt.bfloat16)
            nc.sync.dma_start(
                out=C.xgt2[:, :, :, :],
                in_=C.xg2[:, tok_base + j:, :, :][:, bass.ds(w1, 4), :, :],
            )
    for u in (1, 0):
        if u == 1:
            gA, gB, su = C.g2A, C.g2B, S2
            xgt = C.biasx2_sb if pad2 else C.xgt2
        else:
            gA, gB, su = C.g1A, C.g1B, S1
            xgt = C.xgt1
        for grp, gps in ((0, gA), (1, gB)):
            for m8 in range(8):
                if u == 1 and pad2:
                    moving = C.biasx2_sb[:, grp * 8 + m8, :]
                else:
                    moving = xgt[:, j % 4, grp * 8 + m8, :]
                nc.tensor.matmul(
                    gps[:, m8, 0:su],
                    C.ident_sb[:, :],
                    moving,
                    start=(m8 == 0),
                    stop=False,
                )


def _emit_scan_main(C, j, tok_base, w1, hdst_view):
    """Whh matmuls + cell update for both units; h store every 2nd step.
    hdst_view: dram view [128, HK, nt, S] receiving bf16 h at index w+j."""
    nc, bass, mybir = C.nc, C.bass, C.mybir
    A = mybir.ActivationFunctionType

    for u in (1, 0):  # big unit first
        if u == 1:
            ssl, su, whh = slice(S1, S), S2, C.whh2_sb
            gA, gB, act = C.g2A, C.g2B, C.act2
            tg, t1s, t2s, tcv = C.tg2, C.t12, C.t22, C.tc2
        else:
            ssl, su, whh = slice(0, S1), S1, C.whh1_sb
            gA, gB, act = C.g1A, C.g1B, C.act1
            tg, t1s, t2s, tcv = C.tg1, C.t11, C.t21, C.tc1

        # accumulate Whh@h in fp8 DoubleRow on top of the injected xg
        for grp, gps in ((0, gA), (1, gB)):
            for m8 in range(8):
                for kp in range(2):
                    nc.tensor.matmul(
                        gps[:, m8, 0:su],
                        whh[:, kp, :, grp * 8 + m8, :],
                        C.h8[:, 2 * kp:2 * kp + 2, ssl],
                        start=False,
                        stop=(m8 == 7 and kp == 1),
                        perf_mode=mybir.MatmulPerfMode.DoubleRow,
                    )
        # group A done -> tanh(g) [chunks 0:4], sigmoid(i) [4:8]
        nc.scalar.activation(tg[:, :, :], gA[:, 0:4, 0:su], A.Tanh, scale=1.0 / WS)
        nc.scalar.activation(act[:, 0:4, :], gA[:, 4:8, 0:su], A.Sigmoid, scale=1.0 / WS)
        # group B done -> sigmoid(f,o) [8:16]
        nc.scalar.activation(act[:, 4:12, :], gB[:, :, 0:su], A.Sigmoid, scale=1.0 / WS)
        # c = sig(f)*c + sig(i)*tanh(g);  h = sig(o)*tanh(c)
        nc.vector.tensor_mul(t1s[:, :, :], act[:, 0:4, :], tg[:, :, :])
        nc.vector.tensor_mul(t2s[:, :, :], act[:, 4:8, :], C.c_sb[:, :, ssl])
        nc.vector.tensor_add(C.c_sb[:, :, ssl], t1s[:, :, :], t2s[:, :, :])
        nc.scalar.activation(tcv[:, :, :], C.c_sb[:, :, ssl], A.Tanh)
        nc.vector.tensor_mul(C.h8[:, :, ssl], act[:, 8:12, :], tcv[:, :, :])
        # bf16 h for the handoff / output, from the fp32 operands (NOT from
        # the fp8 state - fp8 noise here would leak into L1 and the output);
        # 4 rotating t-slots so the paired store never stalls the queues
        nc.vector.tensor_mul(C.hbf[:, :, j % 4, ssl], act[:, 8:12, :], tcv[:, :, :])

    if j % 2 == 1:  # store two steps of h per DMA
        sl = (j - 1) % 4
        nc.gpsimd.dma_start(
            out=hdst_view[:, :, j - 1:, :][:, :, bass.ds(w1, 2), :],
            in_=C.hbf[:, :, sl:sl + 2, :],
        )


def build_nc(n_cores=NCORE, unroll=False):
    import bass_rust
    import concourse.bass as bass
    import concourse.mybir as mybir
    from concourse import tile
    from contextlib import ExitStack

    dt = mybir.dt
    nc = bass.Bass("TRN2", target_bir_lowering=False, debug=False,
                   num_devices=(1 if unroll else n_cores))

    C = _Ctx()
    C.nc, C.bass, C.mybir = nc, bass, mybir

    C.xT = nc.dram_tensor("xT", [DK, 128, T, S], dt.bfloat16, kind="ExternalInput").ap()
    C.xT1o = nc.dram_tensor("xT1o", [DK, 128, T, S1], dt.bfloat16, kind="ExternalInput").ap()
    biasx2 = nc.dram_tensor("biasx2", [2, 128, MC, S2], dt.bfloat16, kind="ExternalInput").ap()
    w_in = {}
    for nm in ("A1", "A2", "B1", "B2"):
        w_in[f"wih{nm}"] = nc.dram_tensor(f"wih{nm}", [128, DK, MC, 128], dt.bfloat16, kind="ExternalInput").ap()
        w_in[f"whh{nm}"] = nc.dram_tensor(f"whh{nm}", [128, 2, 2, MC, 128], dt.float8e4, kind="ExternalInput").ap()
        w_in[f"bias{nm}"] = nc.dram_tensor(f"bias{nm}", [128, MC], dt.float32, kind="ExternalInput").ap()
    ident = nc.dram_tensor("ident", [128, 128], dt.bfloat16, kind="ExternalInput").ap()
    flag = nc.dram_tensor("flag", [1, 4], dt.int32, kind="ExternalInput").ap()
    out = nc.dram_tensor("out", [HK, 128, T, S], dt.bfloat16, kind="ExternalOutput").ap()

    C.xg1 = nc.dram_tensor("xg1", [128, T, MC, S1], dt.bfloat16).ap()
    C.xg2 = nc.dram_tensor("xg2", [128, T, MC, S2], dt.bfloat16).ap()
    hT = nc.dram_tensor("hT", [HK, 128, T, S], dt.bfloat16).ap()
    # own h reversed in 64-token quarters: hTr[q] = reverse(hT[T-64(q+1) :
    # T-64q]) = global tokens in descending order; quarter 0 ships first
    # (the partner's L1-IP consumes it first).
    hTr = [nc.dram_tensor(f"hTr{q}", [HK, 128, 64, S], dt.bfloat16).ap()
           for q in range(4)]
    gathR = [nc.dram_tensor(f"gathR{q}", [2, HK, 128, 64, S], dt.bfloat16).ap()
             for q in range(4)]

    SafeTC = _safe_tc(tile, bass_rust)
    groups = [[2 * k, 2 * k + 1] for k in range(max(n_cores // 2, 1))]

    with SafeTC(nc) as tc, ExitStack() as ctx:
        cpool = ctx.enter_context(tc.tile_pool(name="const", bufs=1))
        C.wih1_sb = cpool.tile([128, DK, MC, 128], dt.bfloat16, name="wih1_sb")
        C.wih2_sb = cpool.tile([128, DK, MC, 128], dt.bfloat16, name="wih2_sb")
        C.whh1_sb = cpool.tile([128, 2, 2, MC, 128], dt.float8e4, name="whh1_sb")
        C.whh2_sb = cpool.tile([128, 2, 2, MC, 128], dt.float8e4, name="whh2_sb")
        C.bias1_sb = cpool.tile([128, MC], dt.float32, name="bias1_sb")
        C.bias2_sb = cpool.tile([128, MC], dt.float32, name="bias2_sb")
        C.ident_sb = cpool.tile([128, 128], dt.bfloat16, name="ident_sb")
        C.biasx2A = cpool.tile([128, MC, S2], dt.bfloat16, name="bx2A")
        flag_sb = cpool.tile([1, 4], dt.int32, name="flag_sb")
        nc.gpsimd.dma_start(out=C.biasx2A[...], in_=biasx2[0])

        def load_layer(nm1, nm2):
            engs = [nc.sync, nc.scalar, nc.gpsimd]
            for q, (sb, dr) in enumerate(
                    [(C.wih1_sb, w_in[f"wih{nm1}"]), (C.wih2_sb, w_in[f"wih{nm2}"]),
                     (C.whh1_sb, w_in[f"whh{nm1}"]), (C.whh2_sb, w_in[f"whh{nm2}"]),
                     (C.bias1_sb, w_in[f"bias{nm1}"]), (C.bias2_sb, w_in[f"bias{nm2}"])]):
                engs[q % 3].dma_start(out=sb[...], in_=dr[...])

        load_layer("A1", "A2")
        nc.sync.dma_start(out=C.ident_sb[...], in_=ident[...])
        nc.sync.dma_start(out=flag_sb[...], in_=flag[...])

        if unroll:
            C.vb = 1
        else:
            tmp = nc.alloc_registers("vb_r")
            nc.regs_load(tmp, flag_sb[0:1, 1:2])
            C.vb = nc.snap(tmp, donate=True, min_val=0, max_val=1)

        C.pending_st = []
        spool = ctx.enter_context(tc.tile_pool(name="state", bufs=1))
        C.h8 = spool.tile([128, HK, S], dt.float8e4, name="h8")
        C.hbf = spool.tile([128, HK, 4, S], dt.bfloat16, name="hbf")
        C.c_sb = spool.tile([128, HK, S], dt.float32, name="c_sb")

        def gview(gh, slot):
            """[128, HK, TH, S] view of gath half gh at slot (static)."""
            return gh[slot].rearrange("k p t j -> p k t j")

        def hview(ht):
            return ht.rearrange("k p t j -> p k t j")

        for lay in range(2):
            nm1, nm2 = ("A1", "A2") if lay == 0 else ("B1", "B2")
            with ExitStack() as phase:
                C.rhs_pool = phase.enter_context(tc.tile_pool(name=f"rhs{lay}", bufs=2))
                C.rhs1_pool = phase.enter_context(tc.tile_pool(name=f"rhs1o{lay}", bufs=2))
                C.ip_ps_pool = phase.enter_context(tc.tile_pool(name=f"ipps{lay}", bufs=4, space="PSUM"))
                C.stg1_pool = phase.enter_context(tc.tile_pool(name=f"s1p{lay}", bufs=2))
                C.stg2_pool = phase.enter_context(tc.tile_pool(name=f"s2p{lay}", bufs=2))
                C.xgt1_pool = phase.enter_context(tc.tile_pool(name=f"x1p{lay}", bufs=4))
                C.xgt2_pool = phase.enter_context(tc.tile_pool(name=f"x2p{lay}", bufs=4))
                C.prt_pool = phase.enter_context(tc.tile_pool(name=f"prt{lay}", bufs=2))
                gpool = phase.enter_context(tc.tile_pool(name=f"g{lay}", bufs=1, space="PSUM"))
                apool = phase.enter_context(tc.tile_pool(name=f"act{lay}", bufs=1))

                C.g1A = gpool.tile([128, 8, 64], dt.float32, name=f"g1A{lay}", space="PSUM")
                C.g1B = gpool.tile([128, 8, 64], dt.float32, name=f"g1B{lay}", space="PSUM")
                C.g2A = gpool.tile([128, 8, 64], dt.float32, name=f"g2A{lay}", space="PSUM")
                C.g2B = gpool.tile([128, 8, 64], dt.float32, name=f"g2B{lay}", space="PSUM")
                C.act1 = apool.tile([128, 12, S1], dt.float32, name=f"a1{lay}")
                C.act2 = apool.tile([128, 12, S2], dt.float32, name=f"a2{lay}")
                C.tg1 = apool.tile([128, HK, S1], dt.float32, name=f"tg1{lay}")
                C.tg2 = apool.tile([128, HK, S2], dt.float32, name=f"tg2{lay}")
                C.t11 = apool.tile([128, HK, S1], dt.float32, name=f"t11{lay}")
                C.t12 = apool.tile([128, HK, S2], dt.float32, name=f"t12{lay}")
                C.t21 = apool.tile([128, HK, S1], dt.float32, name=f"t21{lay}")
                C.t22 = apool.tile([128, HK, S2], dt.float32, name=f"t22{lay}")
                C.tc1 = apool.tile([128, HK, S1], dt.float32, name=f"tc1{lay}")
                C.tc2 = apool.tile([128, HK, S2], dt.float32, name=f"tc2{lay}")

                C.u2_pad_from = T2 if lay == 0 else T
                if lay == 1:
                    load_layer(nm1, nm2)
                C.biasx2_sb = C.biasx2A
                nc.vector.memset(C.h8[:, :, :], 0.0)
                nc.vector.memset(C.hbf[:, :, :, :], 0.0)
                nc.vector.memset(C.c_sb[:, :, :], 0.0)

                # segments: (nsteps, tok_base, l1 normal view, l1 rev view,
                #            h-store view, do_ip)
                if lay == 0:
                    segs = [
                        (TH, 0, None, hview(hT), True),
                        (96, TH, None, hview(hT)[:, :, TH:, :], True),
                        (LEAD_T, T - LEAD_T, None,
                         hview(hT)[:, :, T - LEAD_T:, :], False),
                    ]
                else:
                    # partner h (already reversed to global 255-tau order)
                    C.gR_v = [g[bass.ds(C.vb, 1), :, :, :, :][0]
                              .rearrange("k p t j -> p k t j") for g in gathR]
                    C.gran = {}
                    segs = [
                        (96, 0, hview(hT)[:, :, LEAD_T:, :], hview(out), True),
                        (128, 96, hview(hT)[:, :, TH:, :],
                         hview(out)[:, :, 96:, :], True),
                        (LEAD_T, T - LEAD_T, None,
                         hview(out)[:, :, T - LEAD_T:, :], False),
                    ]

                # IP lead groups (static indices; unsliced views)
                if lay == 0:
                    C.l1_src = None
                else:
                    C.l1_src = hview(hT)
                    _emit_granule_load(C, 0)
                for g in range(GLEAD):
                    base = g * GT
                    for j in range(GT):
                        _emit_ip_subchunk(C, j, 0, base, base * S1, base * S2)
                        _emit_ip_sts(C)

                def emit_rev(q):
                    # hTr[q][j] = hT[T - 64q - 1 - j]
                    lo = T - 64 * (q + 1)
                    for kc in range(HK):
                        nc.sync.dma_start(
                            out=hTr[q][kc, :, :, :],
                            in_=_rev_t(hT[kc, :, lo:lo + 64, :], 1),
                        )

                for si, (nsteps, tok_base, l1s, hv, do_ip) in enumerate(segs):
                    C.l1_src = l1s
                    for i in range(0, nsteps, GT):
                        for j in range(GT):
                            _emit_scan_pre(C, j, tok_base, i)
                            if do_ip:
                                _emit_ip_subchunk(
                                    C, j, tok_base + LEAD_T, i, i * S1, i * S2)
                            _emit_scan_main(C, j, tok_base, i, hv)
                            if do_ip:
                                _emit_ip_sts(C)
                    if lay == 0 and si == 0:
                        # first t-half of hT complete: reverse quarters 3,2 now
                        emit_rev(3)
                        emit_rev(2)

                if lay == 0:
                    # quarters 0,1 (global tokens 255..128) depend on the
                    # last scan steps; 2,3 were already reversed after seg0.
                    for q in (1, 0):
                        emit_rev(q)
                    for q in range(4):
                        if unroll:
                            for v in range(2):
                                nc.sync.dma_start(
                                    out=gathR[q][v, :, :, :, :], in_=hTr[q][...])
                        else:
                            nc.gpsimd.collective_compute(
                                "AllGather", mybir.AluOpType.bypass,
                                replica_groups=groups,
                                ins=[hTr[q][...]],
                                outs=[gathR[q][0:2, :, :, :, :]],
                            )
    _split_waits(nc, mybir)
    return nc


# ---------------- host-side data prep ----------------

GATE_PERM = (2, 0, 1, 3)  # new block order [g, i, f, o] from [i, f, g, o]


def _reorder_gates(W):
    """W: (4H, ...) -> gate blocks reordered to [g, i, f, o]."""
    blocks = W.reshape(4, H, *W.shape[1:])
    return np.concatenate([blocks[p] for p in GATE_PERM], axis=0)


def _wih_tiles(W, swap_k_halves=False):
    """(2048, K) fp32 -> [128(kp), K/128, MC, 128(mp)] bf16, x32 scaled.
    swap_k_halves: put columns 512:1024 first (bwd cores' L1 weights: the
    rhs always loads own-direction h into kc 0:4)."""
    Wr = _reorder_gates(W) * WS
    if swap_k_halves:
        Wr = np.concatenate([Wr[:, H:], Wr[:, :H]], axis=1)
    M, K = Wr.shape
    t = Wr.reshape(MC, 128, K // 128, 128)       # [mc, mp, kc, kp]
    return np.ascontiguousarray(t.transpose(3, 2, 0, 1)).astype(BF16)


def _whh_dr_tiles(W):
    """(2048, 512) fp32 -> [128(kp), 2(pair), 2(j), MC, 128(mp)] fp8, x32."""
    Wr = _reorder_gates(W) * WS
    t = Wr.reshape(MC, 128, 4, 128)              # [mc, mp, kchunk, kp]
    t = t.reshape(MC, 128, 2, 2, 128)            # [mc, mp, pair, j, kp]
    return np.ascontiguousarray(t.transpose(4, 2, 3, 0, 1)).astype(FP8)


def _bias_tiles(b):
    br = _reorder_gates(b) * WS
    return np.ascontiguousarray(br.reshape(MC, 128).T.astype(np.float32))


def _core_inputs(u1, u2):
    """u1 (T,S1,D), u2 (T,S2,D) fp32 local time -> xT (DK, 128, T, S) bf16."""
    x48 = np.concatenate([u1, u2], axis=1)          # (T, S, D)
    xt = x48.transpose(2, 0, 1)                     # (D, T, S)
    return np.ascontiguousarray(xt.reshape(DK, 128, T, S)).astype(BF16)


def _prep_inputs(inputs):
    U = np.asarray(inputs["U"], np.float32)            # (T, B, D)
    qmask = np.asarray(inputs["qmask"], np.float32)    # (B, T, P)
    U_bt = U.transpose(1, 0, 2)
    mask = qmask > 0
    pos = np.cumsum(mask.astype(np.int64), axis=1) - 1

    parties = np.zeros((P, B, T, D), np.float32)
    for p in range(P):
        b_idx, t_idx = np.nonzero(mask[:, :, p])
        parties[p, b_idx, pos[b_idx, t_idx, p]] = U_bt[b_idx, t_idx]
    partiesM = parties.reshape(P * B, T, D).transpose(1, 0, 2)  # (T, 128, D)
    assert int(mask.sum(axis=1).max()) < T2, (
        "party stream longer than the hardcoded all-pad cutoff T2"
    )

    def wset(stack, lay, d):
        return (
            _wih_tiles(np.asarray(inputs[f"{stack}_Wih{lay}"][d], np.float32),
                       swap_k_halves=(lay == 1 and d == 1)),
            _whh_dr_tiles(np.asarray(inputs[f"{stack}_Whh{lay}"][d], np.float32)),
            _bias_tiles(np.asarray(inputs[f"{stack}_b{lay}"][d], np.float32)),
        )

    wsets = {(st, la, d): wset(st, la, d)
             for st in ("rnn", "rnnp") for la in (0, 1) for d in (0, 1)}
    ident_np = np.eye(128, dtype=BF16)

    in_maps = []
    for c in range(NCORE):
        k, d = c // 2, c % 2
        u1 = U[:, 16 * k:16 * k + 16, :]
        u2 = partiesM[:, 32 * k:32 * k + 32, :]
        if d == 1:
            u1, u2 = u1[::-1], u2[::-1]
        b2 = wsets[("rnnp", 0, d)][2]  # [128, MC] fp32, x32 scaled
        biasx2 = np.zeros((2, 128, MC, S2), BF16)
        biasx2[0] = np.repeat(b2.astype(BF16)[:, :, None], S2, axis=2)
        m = {
            "xT": _core_inputs(u1, u2),
            "xT1o": np.ascontiguousarray(
                u1.transpose(2, 0, 1).reshape(DK, 128, T, S1)).astype(BF16),
            "biasx2": biasx2,
            "ident": ident_np,
            # [unused, partner-slot, cond_slot0=(slot==0), cond_slot1]
            "flag": np.array([[0, 1 - d, d, 1 - d]], np.int32),
        }
        for la, nm in ((0, "A"), (1, "B")):
            for ui, st in ((1, "rnn"), (2, "rnnp")):
                wih, whh, bias = wsets[(st, la, d)]
                m[f"wih{nm}{ui}"] = wih
                m[f"whh{nm}{ui}"] = whh
                m[f"bias{nm}{ui}"] = bias
        in_maps.append(m)
    return in_maps, mask, pos


def _assemble(results, mask, pos):
    # per-core out: (HK, 128, T, S) bf16; feature dim on (HK,128)=512
    o = []
    for c in range(NCORE):
        oc = np.asarray(results[c]["out"]).astype(np.float32)
        oc = oc.reshape(H, T, S).transpose(1, 2, 0)    # (T, S, 512) local time
        if c % 2 == 1:
            oc = oc[::-1]
        o.append(oc)

    U_s = np.zeros((T, B, 2 * H), np.float32)
    E = np.zeros((P, B, T, 2 * H), np.float32)
    for k in range(4):
        fwd, bwd = o[2 * k], o[2 * k + 1]
        U_s[:, 16 * k:16 * k + 16, 0:H] = fwd[:, 0:S1]
        U_s[:, 16 * k:16 * k + 16, H:2 * H] = bwd[:, 0:S1]
        for i in range(S2):
            ms = 32 * k + i
            p, b = divmod(ms, B)
            E[p, b, :, 0:H] = fwd[:, S1 + i]
            E[p, b, :, H:2 * H] = bwd[:, S1 + i]

    U_p = np.zeros((B, T, 2 * H), np.float32)
    for p in range(P):
        idx = np.clip(pos[:, :, p], 0, T - 1)
        gathered = np.take_along_axis(E[p], idx[:, :, None], axis=1)
        U_p = np.where(mask[:, :, p][:, :, None], gathered, U_p)
    U_p = U_p.transpose(1, 0, 2)
    return np.concatenate([U_s, U_p], axis=-1).astype(np.float32)


def _get_compiled():
    if "nc" not in _CACHE:
        _CACHE["nc"] = build_nc()
    return _CACHE["nc"]


def kernel(**inputs):
    from concourse.bass_utils import run_bass_kernel_spmd

    nc = _get_compiled()
    in_maps, mask, pos = _prep_inputs(inputs)
    trace = bool(int(os.environ.get("KERNEL_TRACE", "0")))
    res = run_bass_kernel_spmd(nc, in_maps, list(range(NCORE)), trace=trace)
    _CACHE["last_exec_time_ns"] = res.exec_time_ns
    return _assemble(res.results, mask, pos)
